# revision 33
# baseline (speedup 1.0000x reference)
"""Trainium2 Bass kernel for a causal self-attention block (GQA + gated value
embedding + RoPE + QK-RMSNorm), sharded over 8 NeuronCores.

Sharding: 8 cores = 2 (batch) x 4 (kv-head groups).  Each core computes, for
its batch b and head-group g (4 q-heads + 1 kv-head):
    q/k/v projections, gated ve addition, RoPE, RMSNorm, causal attention,
    and the partial output projection  y_g @ Wproj[g*512:(g+1)*512, :].
The host sums the 4 per-group partials for each batch (the Wproj
contraction distributes over head groups).

x is transposed on the host (xT), so the qkv matmuls consume it directly as
the stationary operand with no on-chip transposes.  The softmax denominator
is accumulated through an all-ones [128,128] stationary matmul, which makes
the PSUM result partition-replicated so the normalization needs no
cross-partition broadcast.  The output projection is interleaved into the
attention loop chunk-by-chunk to keep the PE warm.

Self-contained: hardcodes shapes; accepts FULL inputs, returns FULL output.
"""

from collections import deque
from contextlib import ExitStack

import numpy as np

import concourse.bacc as bacc
import concourse.bass as bass
import concourse.mybir as mybir
import concourse.tile as tile
from concourse.bass_utils import run_bass_kernel_spmd
from concourse.masks import make_identity

F32 = mybir.dt.float32
F32R = mybir.dt.float32r
BF16 = mybir.dt.bfloat16
I32 = mybir.dt.int32
AF = mybir.ActivationFunctionType
ALU = mybir.AluOpType
AX = mybir.AxisListType

B, C, HD, NHL, GC = 2, 2048, 128, 4, 32  # NHL = local q heads per core
EPS = float(np.finfo(np.float32).eps)
ISQ = 1.0 / float(np.sqrt(128.0))
RSQRT_MAGIC = 0x5F3759DF


def _bcast(ap_, idx, count):
    """Insert a step-0 (broadcast) dim at position idx of the AP dims."""
    lst = [list(p) for p in ap_.ap]
    lst.insert(idx, [0, count])
    return bass.AP(ap_.tensor, ap_.offset, lst)


def build(T=2048):
    TB = T // 128   # token blocks
    CT = C // 128   # contraction tiles for qkv
    NCH = T // 512  # i-chunks for attention
    OC = C // 512   # output chunks for proj

    nc = bacc.Bacc("TRN2", target_bir_lowering=False, debug=False)
    # all inputs host-tiled to [128 partitions, ...contiguous] so every DMA
    # needs only 128 descriptors (one 4-16KB segment per partition)
    xt = nc.dram_tensor("xt", [TB, 128, CT, 128], BF16, kind="ExternalInput")
    xg = nc.dram_tensor("xg", [128, TB, GC], BF16, kind="ExternalInput")
    wq = nc.dram_tensor("wq", [128, CT, NHL * HD], BF16, kind="ExternalInput")
    wkv = nc.dram_tensor("wkv", [128, CT, 2 * HD], BF16, kind="ExternalInput")
    wproj = nc.dram_tensor("wproj", [128, NHL, OC, 512], BF16,
                           kind="ExternalInput")
    wgate = nc.dram_tensor("wgate", [GC, 1], F32, kind="ExternalInput")
    ve2 = nc.dram_tensor("ve2", [128, TB, HD], BF16, kind="ExternalInput")
    cosn = nc.dram_tensor("cosn", [128, TB, 64], BF16, kind="ExternalInput")
    sinn = nc.dram_tensor("sinn", [128, TB, 64], BF16, kind="ExternalInput")
    out = nc.dram_tensor("out", [T, C], F32, kind="ExternalOutput")

    with ExitStack() as stk:
        tc = stk.enter_context(tile.TileContext(nc))
        gpool = stk.enter_context(tc.tile_pool(name="gconst", bufs=1))
        identw = gpool.tile([128, 128], F32)
        nc.gpsimd.memset(identw, 0.0)
        ones_g = gpool.tile([128, 1], F32)
        nc.gpsimd.memset(ones_g, 1.0)
        nc.gpsimd.affine_select(
            out=identw, in_=_bcast(ones_g[:, 0], 1, 128), pattern=[[1, 128]],
            compare_op=ALU.is_equal, fill=0.0, base=0, channel_multiplier=-1)
        ident = gpool.tile([128, 128], F32)
        make_identity(nc, ident)
        identr = gpool.tile([128, 128], F32R)
        nc.vector.tensor_copy(out=identr, in_=ident)
        ones_f = gpool.tile([128, 512], F32)
        ones128 = gpool.tile([128, 128], F32R)
        mask_f = gpool.tile([128, 4, 512], F32)
        masks = gpool.tile([128, 4, 512], F32R)

        def build_masks():
            # static causal masks for the 4 diagonal positions: mask[r][p,col]
            # = 1 iff col >= 128*r + p (col = q index within its 512-chunk,
            # p = k%128).  Emitted after phase A's DMA issues so the gpsimd
            # queue serves the weight DMAs first.
            nc.vector.memset(ones_f, 1.0)
            nc.vector.tensor_copy(out=ones128, in_=ones_f[:, 0:128])
            nc.vector.memset(mask_f, 0.0)
            for r in range(4):
                nc.gpsimd.affine_select(
                    out=mask_f[:, r, 128 * r:512],
                    in_=ones_f[:, 0:512 - 128 * r],
                    pattern=[[1, 512 - 128 * r]], compare_op=ALU.is_ge,
                    fill=0.0, base=0, channel_multiplier=-1)
            nc.vector.tensor_copy(out=masks, in_=mask_f)

        # PE warmup: dummy transposes so HAM reaches full clock while the
        # first DMAs land.
        with tc.tile_pool(name="warm", bufs=2, space="PSUM") as warm:
            for _ in range(12):
                w_ps = warm.tile([128, 128], F32R, tag="wps", name="wps")
                nc.tensor.transpose(w_ps, identr, identr)

        persist = stk.enter_context(tc.tile_pool(name="persist", bufs=1))
        qT = persist.tile([128, NHL, T], BF16)   # [d, h, t]
        kT = persist.tile([128, T], BF16)        # [d, t]
        vS = persist.tile([128, TB, HD], F32R)   # [t%128, t//128, d]
        yT = persist.tile([128, NHL, T], BF16)   # [d, h, t]

        # ---------------- phase A: qkv + rope + rmsnorm --------------------
        with nc.named_scope("phaseA"), \
                tc.tile_pool(name="wA", bufs=1) as wA, \
                tc.tile_pool(name="xA", bufs=4) as xA, \
                tc.tile_pool(name="sbA", bufs=1) as sbA, \
                tc.tile_pool(name="qkh", bufs=5) as qkh, \
                tc.tile_pool(name="psq", bufs=2, space="PSUM") as psq, \
                tc.tile_pool(name="pskv", bufs=2, space="PSUM") as pskv, \
                tc.tile_pool(name="pst", bufs=4, space="PSUM") as pst:
            chunks = {}

            def load_chunk(tb):
                t = xA.tile([128, CT, 128], BF16, tag="xch", name="xch")
                nc.sync.dma_start(out=t, in_=xt[tb])
                chunks[tb] = t

            # small per-token tensors at the head of the sync queue so they
            # land before the bulk x/weight traffic
            wgb_sb = wA.tile([128, GC], F32)
            nc.sync.dma_start(out=wgb_sb, in_=_bcast(wgate[:, 0], 0, 128))
            xg_sb = wA.tile([128, TB, GC], BF16)
            nc.sync.dma_start(out=xg_sb, in_=xg[:])
            cos_sb = wA.tile([128, TB, 64], BF16)
            nc.sync.dma_start(out=cos_sb, in_=cosn[:])
            sin_sb = wA.tile([128, TB, 64], BF16)
            nc.sync.dma_start(out=sin_sb, in_=sinn[:])
            ve_sb = wA.tile([128, TB, HD], BF16)
            nc.sync.dma_start(out=ve_sb, in_=ve2[:])
            wq_sb = wA.tile([128, CT, NHL * HD], BF16)
            wkv_sb = wA.tile([128, CT, 2 * HD], BF16)
            nc.gpsimd.dma_start(out=wq_sb, in_=wq[:])
            nc.gpsimd.dma_start(out=wkv_sb, in_=wkv[:])
            load_chunk(0)
            load_chunk(1)
            build_masks()
            load_chunk(2)
            load_chunk(3)

            pend = deque()  # (qhat, khat, t0) awaiting transpose into qT/kT

            def emit_transposes(pqh, pkh, pt0):
                for hh in range(NHL):
                    tq_ps = pst.tile([128, 128], F32R, tag="tps", name="tps")
                    nc.tensor.transpose(
                        tq_ps, pqh[:, hh * 128:(hh + 1) * 128], identr)
                    nc.scalar.copy(out=qT[:, hh, pt0:pt0 + 128], in_=tq_ps)
                tk_ps = pst.tile([128, 128], F32R, tag="tps", name="tps")
                nc.tensor.transpose(tk_ps, pkh, identr)
                nc.scalar.copy(out=kT[:, pt0:pt0 + 128], in_=tk_ps)

            for tb in range(TB):
                t0 = tb * 128
                if tb + 4 < TB:
                    load_chunk(tb + 4)
                xcht = chunks.pop(tb)
                q_ps = psq.tile([128, NHL * HD], F32, tag="qps")
                kv_ps = pskv.tile([128, 2 * HD], F32, tag="kvps")
                for ct in range(CT):
                    xts = xcht[:, ct, :]
                    nc.tensor.matmul(
                        q_ps, lhsT=xts, rhs=wq_sb[:, ct, :],
                        start=(ct == 0), stop=(ct == CT - 1))
                    nc.tensor.matmul(
                        kv_ps, lhsT=xts, rhs=wkv_sb[:, ct, :],
                        start=(ct == 0), stop=(ct == CT - 1))

                # four-blocks-ago qhat/khat -> qT/kT (PE transposes)
                if len(pend) >= 4:
                    emit_transposes(*pend.popleft())

                # free the PSUM banks right away: q/kv -> SBUF on the
                # scalar engine so the next block's matmuls never wait
                qsb = sbA.tile([128, NHL * HD], F32, tag="qsb")
                nc.scalar.copy(out=qsb, in_=q_ps)
                kvsb = sbA.tile([128, 2 * HD], F32, tag="kvsb")
                nc.scalar.copy(out=kvsb, in_=kv_ps)
                # gate = sigmoid(x[:, :32] @ wgate);  v = v_mm + gate * (2*ve)
                zg_sb = sbA.tile([128, 1], F32, tag="zg")
                zscr = sbA.tile([128, GC], F32, tag="zscr")
                nc.vector.scalar_tensor_tensor(
                    out=zscr, in0=xg_sb[:, tb, :], scalar=1.0, in1=wgb_sb,
                    op0=ALU.bypass, op1=ALU.mult, accum_out=zg_sb)
                g_sb = sbA.tile([128, 1], F32, tag="gsb")
                nc.scalar.activation(g_sb, zg_sb, AF.Sigmoid)
                nc.vector.scalar_tensor_tensor(
                    out=vS[:, tb, :], in0=ve_sb[:, tb, :], scalar=g_sb,
                    in1=kvsb[:, HD:2 * HD], op0=ALU.mult, op1=ALU.add)

                # ---- RoPE on q (4 heads batched) and k ----
                cosB = _bcast(cos_sb[:, tb, :], 1, NHL)
                sinB = _bcast(sin_sb[:, tb, :], 1, NHL)
                qv = qsb.rearrange("p (h d) -> p h d", h=NHL)
                qh = sbA.tile([128, NHL * HD], F32, tag="qh")
                qhv = qh.rearrange("p (h d) -> p h d", h=NHL)
                tmp = sbA.tile([128, NHL, 64], F32, tag="tmp")
                nc.vector.tensor_tensor(
                    out=qhv[:, :, 0:64], in0=qv[:, :, 0:64], in1=cosB, op=ALU.mult)
                nc.vector.tensor_tensor(
                    out=tmp, in0=qv[:, :, 64:128], in1=sinB, op=ALU.mult)
                nc.vector.tensor_tensor(
                    out=qhv[:, :, 0:64], in0=qhv[:, :, 0:64], in1=tmp, op=ALU.add)
                nc.vector.tensor_tensor(
                    out=qhv[:, :, 64:128], in0=qv[:, :, 64:128], in1=cosB, op=ALU.mult)
                nc.vector.tensor_tensor(
                    out=tmp, in0=qv[:, :, 0:64], in1=sinB, op=ALU.mult)
                nc.vector.tensor_tensor(
                    out=qhv[:, :, 64:128], in0=qhv[:, :, 64:128], in1=tmp,
                    op=ALU.subtract)
                kv = kvsb[:, 0:HD]
                kh = sbA.tile([128, HD], F32, tag="kh")
                ktmp = sbA.tile([128, 64], F32, tag="ktmp")
                cs1 = cos_sb[:, tb, :]
                sn1 = sin_sb[:, tb, :]
                nc.vector.tensor_tensor(
                    out=kh[:, 0:64], in0=kv[:, 0:64], in1=cs1, op=ALU.mult)
                nc.vector.tensor_tensor(
                    out=ktmp, in0=kv[:, 64:128], in1=sn1, op=ALU.mult)
                nc.vector.tensor_tensor(
                    out=kh[:, 0:64], in0=kh[:, 0:64], in1=ktmp, op=ALU.add)
                nc.vector.tensor_tensor(
                    out=kh[:, 64:128], in0=kv[:, 64:128], in1=cs1, op=ALU.mult)
                nc.vector.tensor_tensor(
                    out=ktmp, in0=kv[:, 0:64], in1=sn1, op=ALU.mult)
                nc.vector.tensor_tensor(
                    out=kh[:, 64:128], in0=kh[:, 64:128], in1=ktmp, op=ALU.subtract)

                # ---- RMSNorm scales for q heads + k in one [128, 5] batch ----
                sq2 = sbA.tile([128, NHL * HD], F32, tag="sq2")
                red = sbA.tile([128, NHL + 1], F32, tag="red")
                for h5 in range(NHL):
                    nc.vector.scalar_tensor_tensor(
                        out=sq2[:, h5 * HD:(h5 + 1) * HD],
                        in0=qh[:, h5 * HD:(h5 + 1) * HD], scalar=1.0,
                        in1=qh[:, h5 * HD:(h5 + 1) * HD], op0=ALU.bypass,
                        op1=ALU.mult, accum_out=red[:, h5:h5 + 1])
                ksq = sbA.tile([128, HD], F32, tag="ksq")
                nc.vector.scalar_tensor_tensor(
                    out=ksq, in0=kh, scalar=1.0, in1=kh, op0=ALU.bypass,
                    op1=ALU.mult, accum_out=red[:, NHL:NHL + 1])
                # m = mean + eps;  rsqrt(m) via bit-trick seed + 2 Newton steps
                # (all on DVE: keeps the scalar engine on a single act table)
                nc.vector.tensor_scalar(
                    out=red, in0=red, scalar1=1.0 / 128.0, scalar2=EPS,
                    op0=ALU.mult, op1=ALU.add)
                rq = sbA.tile([128, NHL + 1], F32, tag="rq")
                rqi = rq.bitcast(I32)
                nc.vector.tensor_scalar(
                    out=rqi, in0=red.bitcast(I32), scalar1=1, scalar2=None,
                    op0=ALU.logical_shift_right)
                nc.vector.tensor_scalar(
                    out=rqi, in0=rqi, scalar1=-1, scalar2=RSQRT_MAGIC,
                    op0=ALU.mult, op1=ALU.add)
                nt = sbA.tile([128, NHL + 1], F32, tag="nt")
                for _ in range(1):
                    nc.vector.tensor_tensor(out=nt, in0=rq, in1=rq, op=ALU.mult)
                    nc.vector.tensor_tensor(out=nt, in0=nt, in1=red, op=ALU.mult)
                    nc.vector.tensor_scalar(
                        out=nt, in0=nt, scalar1=-0.5, scalar2=1.5,
                        op0=ALU.mult, op1=ALU.add)
                    nc.vector.tensor_tensor(out=rq, in0=rq, in1=nt, op=ALU.mult)

                qhat = qkh.tile([128, NHL * HD], F32R, tag="qhat")
                for h5 in range(NHL):
                    nc.gpsimd.tensor_scalar_mul(
                        qhat[:, h5 * HD:(h5 + 1) * HD],
                        qh[:, h5 * HD:(h5 + 1) * HD], rq[:, h5:h5 + 1])
                khat = qkh.tile([128, HD], F32R, tag="khat")
                nc.gpsimd.tensor_scalar_mul(khat, kh, rq[:, NHL:NHL + 1])
                pend.append((qhat, khat, t0))

            while pend:
                emit_transposes(*pend.popleft())

        # ---------------- phase B+C: attention + output projection ---------
        wC = stk.enter_context(tc.tile_pool(name="wC", bufs=1))
        wp_sb = wC.tile([128, NHL, OC, 512], BF16)
        nc.sync.dma_start(out=wp_sb, in_=wproj[:])

        with nc.named_scope("phaseBC"), \
                tc.tile_pool(name="ptB", bufs=4) as ptB, \
                tc.tile_pool(name="smB", bufs=2) as smB, \
                tc.tile_pool(name="sbC", bufs=2) as sbC, \
                tc.tile_pool(name="psS", bufs=2, space="PSUM") as psS, \
                tc.tile_pool(name="psy", bufs=2, space="PSUM") as psy, \
                tc.tile_pool(name="psd", bufs=2, space="PSUM") as psd, \
                tc.tile_pool(name="psC", bufs=2, space="PSUM") as psC:

            av_pend = deque()  # closures: AV+den matmuls (and chunk tails)

            def drain(n):
                while len(av_pend) > n:
                    av_pend.popleft()()

            def emit_c_chunk(c):
                drain(0)
                for tb in range(4 * c, 4 * c + 4):
                    t0 = tb * 128
                    o_sb = sbC.tile([128, C], F32, tag="osb", name="osb")
                    for oc in range(OC):
                        o_ps = psC.tile([128, 512], F32, tag="ops", name="ops")
                        for hh in range(NHL):
                            nc.tensor.matmul(
                                o_ps, lhsT=yT[:, hh, t0:t0 + 128],
                                rhs=wp_sb[:, hh, oc, :],
                                start=(hh == 0), stop=(hh == NHL - 1))
                        dst = o_sb[:, oc * 512:(oc + 1) * 512]
                        if oc % 2 == 0:
                            nc.scalar.copy(out=dst, in_=o_ps)
                        else:
                            nc.vector.tensor_copy(out=dst, in_=o_ps)
                        if oc == 1:
                            nc.sync.dma_start(
                                out=out[t0:t0 + 128, 0:1024], in_=o_sb[:, 0:1024])
                    nc.gpsimd.dma_start(
                        out=out[t0:t0 + 128, 1024:2048], in_=o_sb[:, 1024:2048])

            for c in range(NCH):
                i0 = c * 512
                for hh in range(NHL):
                    if hh == 1 and c >= 1:
                        emit_c_chunk(c - 1)
                    yps = psy.tile([128, 512], F32, tag="yps", name="yps")
                    dps = psd.tile([128, 512], F32, tag="dps", name="dps")
                    njb = 4 * c + 4
                    for jb in range(njb):
                        r = jb - 4 * c
                        io2 = 0 if r < 0 else min(128 * r, 256)
                        sps = psS.tile([128, 512], F32, tag="sps", name="sps")
                        nc.tensor.matmul(
                            sps[:, io2:512],
                            lhsT=kT[:, jb * 128:(jb + 1) * 128],
                            rhs=qT[:, hh, i0 + io2:i0 + 512],
                            start=True, stop=True)
                        pt = ptB.tile([128, 512], F32R, tag="pt", name="pt")
                        nc.scalar.activation(
                            pt[:, io2:512], sps[:, io2:512], AF.Exp, scale=ISQ)
                        if r >= 0:
                            nc.vector.tensor_tensor(
                                out=pt[:, io2:512], in0=pt[:, io2:512],
                                in1=masks[:, r, io2:512], op=ALU.mult)

                        def av(jb=jb, pt=pt, io2=io2, yps=yps, dps=dps,
                               first=(jb == 0), last=(jb == njb - 1),
                               hh=hh, i0=i0):
                            nc.tensor.matmul(
                                yps[:, io2:512], lhsT=vS[:, jb, :],
                                rhs=pt[:, io2:512], start=first, stop=last)
                            nc.tensor.matmul(
                                dps[:, io2:512], lhsT=ones128,
                                rhs=pt[:, io2:512], start=first, stop=last)
                            if last:
                                rcp = smB.tile([128, 512], F32, tag="rcp",
                                               name="rcp")
                                nc.vector.reciprocal_approx_fast(
                                    out=rcp, in_=dps)
                                nc.vector.tensor_tensor(
                                    out=yT[:, hh, i0:i0 + 512], in0=yps,
                                    in1=rcp, op=ALU.mult)

                        av_pend.append(av)
                        drain(2)
            drain(0)
            emit_c_chunk(NCH - 1)

    nc.compile()
    return nc


_NC_CACHE = {}


def get_nc(T=2048):
    if T not in _NC_CACHE:
        _NC_CACHE[T] = build(T)
    return _NC_CACHE[T]


def make_in_maps(x, ve, cos, sin, Wq, Wk, Wv, Wproj, Wgate):
    """Shard full inputs into 8 per-core input maps (2 batch x 4 head groups)."""
    import ml_dtypes
    bf16 = ml_dtypes.bfloat16
    x = np.asarray(x, np.float32)
    ve = np.asarray(ve, np.float32)
    cosn = np.asarray(cos, np.float32)[0, :, 0, :]
    sinn = np.asarray(sin, np.float32)[0, :, 0, :]
    TT = cosn.shape[0]
    cosn = np.ascontiguousarray(
        cosn.astype(bf16).reshape(TT // 128, 128, 64).transpose(1, 0, 2))
    sinn = np.ascontiguousarray(
        sinn.astype(bf16).reshape(TT // 128, 128, 64).transpose(1, 0, 2))
    Wq = np.asarray(Wq, np.float32)
    Wk = np.asarray(Wk, np.float32)
    Wv = np.asarray(Wv, np.float32)
    Wproj = np.asarray(Wproj, np.float32)
    Wgate = np.asarray(Wgate, np.float32)
    T = x.shape[1]
    TB, CT = T // 128, C // 128
    # [TB, p, ct, t] tiling of x (partition p = channel within ct-block)
    xts = [np.ascontiguousarray(
        x[b].astype(bf16).reshape(TB, 128, CT, 128).transpose(0, 3, 2, 1))
        for b in range(B)]
    xgs = [np.ascontiguousarray(
        x[b][:, :GC].astype(bf16).reshape(TB, 128, GC).transpose(1, 0, 2))
        for b in range(B)]
    in_maps = []
    for core in range(8):
        b, g = divmod(core, 4)
        in_maps.append({
            "xt": xts[b],
            "xg": xgs[b],
            "wq": np.ascontiguousarray(
                Wq[:, g * 512:(g + 1) * 512].astype(bf16)
                .reshape(CT, 128, 512).transpose(1, 0, 2)),
            "wkv": np.ascontiguousarray(np.concatenate(
                [Wk[:, g * 128:(g + 1) * 128].astype(bf16)
                 .reshape(CT, 128, 128),
                 Wv[:, g * 128:(g + 1) * 128].astype(bf16)
                 .reshape(CT, 128, 128)], axis=2).transpose(1, 0, 2)),
            "wproj": np.ascontiguousarray(
                Wproj[g * 512:(g + 1) * 512, :].astype(bf16)
                .reshape(4, 128, 4, 512).transpose(1, 0, 2, 3)),
            "wgate": np.ascontiguousarray(Wgate[:, g:g + 1]),
            "ve2": np.ascontiguousarray(
                (2.0 * ve[b][:, g * 128:(g + 1) * 128]).astype(bf16)
                .reshape(TB, 128, 128).transpose(1, 0, 2)),
            "cosn": cosn,
            "sinn": sinn,
        })
    return in_maps


def run_cores(in_maps, trace=False, **kw):
    nc = get_nc(in_maps[0]["xt"].shape[0] * 128)
    return run_bass_kernel_spmd(nc, in_maps, core_ids=list(range(8)), trace=trace, **kw)


def kernel(**inputs):
    in_maps = make_in_maps(
        inputs["x"], inputs["ve"], inputs["cos"], inputs["sin"],
        inputs["Wq"], inputs["Wk"], inputs["Wv"], inputs["Wproj"], inputs["Wgate"])
    res = run_cores(in_maps)
    parts = [res.results[i]["out"] for i in range(8)]
    out = np.stack([
        parts[0] + parts[1] + parts[2] + parts[3],
        parts[4] + parts[5] + parts[6] + parts[7],
    ]).astype(np.float32)
    return out


# revision 35
# speedup vs baseline: 1.4308x; 1.4308x over previous
"""Trainium2 Bass kernel for a causal self-attention block (GQA + gated value
embedding + RoPE + QK-RMSNorm), sharded over 8 NeuronCores.

Sharding: 8 cores = 2 (batch) x 4 (kv-head groups).  Each core computes, for
its batch b and head-group g (4 q-heads + 1 kv-head):
    q/k/v projections, gated ve addition, RoPE, RMSNorm, causal attention,
    and the partial output projection  y_g @ Wproj[g*512:(g+1)*512, :].
The host sums the 4 per-group partials for each batch (the Wproj
contraction distributes over head groups).

x is transposed on the host (xT), so the qkv matmuls consume it directly as
the stationary operand with no on-chip transposes.  The softmax denominator
is accumulated through an all-ones [128,128] stationary matmul, which makes
the PSUM result partition-replicated so the normalization needs no
cross-partition broadcast.  The output projection is interleaved into the
attention loop chunk-by-chunk to keep the PE warm.

Self-contained: hardcodes shapes; accepts FULL inputs, returns FULL output.
"""

from collections import deque
from contextlib import ExitStack

import numpy as np

import concourse.bacc as bacc
import concourse.bass as bass
import concourse.mybir as mybir
import concourse.tile as tile
from concourse.bass_utils import run_bass_kernel_spmd
from concourse.masks import make_identity

F32 = mybir.dt.float32
F32R = mybir.dt.float32r
BF16 = mybir.dt.bfloat16
I32 = mybir.dt.int32
AF = mybir.ActivationFunctionType
ALU = mybir.AluOpType
AX = mybir.AxisListType

B, C, HD, NHL, GC = 2, 2048, 128, 4, 32  # NHL = local q heads per core
EPS = float(np.finfo(np.float32).eps)
ISQ = 1.0 / float(np.sqrt(128.0))
RSQRT_MAGIC = 0x5F3759DF


def _bcast(ap_, idx, count):
    """Insert a step-0 (broadcast) dim at position idx of the AP dims."""
    lst = [list(p) for p in ap_.ap]
    lst.insert(idx, [0, count])
    return bass.AP(ap_.tensor, ap_.offset, lst)


def build(T=2048):
    TB = T // 128   # token blocks
    CT = C // 128   # contraction tiles for qkv
    NCH = T // 512  # i-chunks for attention
    OC = C // 512   # output chunks for proj

    nc = bacc.Bacc("TRN2", target_bir_lowering=False, debug=False)
    # all inputs host-tiled to [128 partitions, ...contiguous] so every DMA
    # needs only 128 descriptors (one 4-16KB segment per partition)
    xt = nc.dram_tensor("xt", [TB, 128, CT, 128], BF16, kind="ExternalInput")
    xg = nc.dram_tensor("xg", [128, TB, GC], BF16, kind="ExternalInput")
    wq = nc.dram_tensor("wq", [128, CT, NHL * HD], BF16, kind="ExternalInput")
    wkv = nc.dram_tensor("wkv", [128, CT, 2 * HD], BF16, kind="ExternalInput")
    wproj = nc.dram_tensor("wproj", [128, NHL, OC, 512], BF16,
                           kind="ExternalInput")
    wgate = nc.dram_tensor("wgate", [GC, 1], F32, kind="ExternalInput")
    ve2 = nc.dram_tensor("ve2", [128, TB, HD], BF16, kind="ExternalInput")
    cosn = nc.dram_tensor("cosn", [128, TB, 64], BF16, kind="ExternalInput")
    sinn = nc.dram_tensor("sinn", [128, TB, 64], BF16, kind="ExternalInput")
    out = nc.dram_tensor("out", [T, C], F32, kind="ExternalOutput")

    with ExitStack() as stk:
        tc = stk.enter_context(tile.TileContext(nc))
        gpool = stk.enter_context(tc.tile_pool(name="gconst", bufs=1))
        identw = gpool.tile([128, 128], F32)
        nc.gpsimd.memset(identw, 0.0)
        ones_g = gpool.tile([128, 1], F32)
        nc.gpsimd.memset(ones_g, 1.0)
        nc.gpsimd.affine_select(
            out=identw, in_=_bcast(ones_g[:, 0], 1, 128), pattern=[[1, 128]],
            compare_op=ALU.is_equal, fill=0.0, base=0, channel_multiplier=-1)
        ident = gpool.tile([128, 128], F32)
        make_identity(nc, ident)
        identr = gpool.tile([128, 128], F32R)
        nc.vector.tensor_copy(out=identr, in_=ident)
        ones_f = gpool.tile([128, 512], F32)
        ones128 = gpool.tile([128, 128], F32R)
        mask_f = gpool.tile([128, 4, 512], F32)
        masks = gpool.tile([128, 4, 512], F32R)

        def build_masks():
            # static causal masks for the 4 diagonal positions: mask[r][p,col]
            # = 1 iff col >= 128*r + p (col = q index within its 512-chunk,
            # p = k%128).  Emitted after phase A's DMA issues so the gpsimd
            # queue serves the weight DMAs first.
            nc.vector.memset(ones_f, 1.0)
            nc.vector.tensor_copy(out=ones128, in_=ones_f[:, 0:128])
            nc.vector.memset(mask_f, 0.0)
            for r in range(4):
                nc.gpsimd.affine_select(
                    out=mask_f[:, r, 128 * r:512],
                    in_=ones_f[:, 0:512 - 128 * r],
                    pattern=[[1, 512 - 128 * r]], compare_op=ALU.is_ge,
                    fill=0.0, base=0, channel_multiplier=-1)
            nc.vector.tensor_copy(out=masks, in_=mask_f)

        # PE warmup: dummy transposes so HAM reaches full clock while the
        # first DMAs land.
        with tc.tile_pool(name="warm", bufs=2, space="PSUM") as warm:
            for _ in range(12):
                w_ps = warm.tile([128, 128], F32R, tag="wps", name="wps")
                nc.tensor.transpose(w_ps, identr, identr)

        persist = stk.enter_context(tc.tile_pool(name="persist", bufs=1))
        qT = persist.tile([128, NHL, T], BF16)   # [d, h, t]
        kT = persist.tile([128, T], BF16)        # [d, t]
        vS = persist.tile([128, TB, HD], F32R)   # [t%128, t//128, d]
        yT = persist.tile([128, NHL, T], BF16)   # [d, h, t]

        # ---------------- phase A: qkv + rope + rmsnorm --------------------
        with nc.named_scope("phaseA"), \
                tc.tile_pool(name="wA", bufs=1) as wA, \
                tc.tile_pool(name="xA", bufs=4) as xA, \
                tc.tile_pool(name="sbA", bufs=1) as sbA, \
                tc.tile_pool(name="qkh", bufs=5) as qkh, \
                tc.tile_pool(name="psq", bufs=2, space="PSUM") as psq, \
                tc.tile_pool(name="pskv", bufs=2, space="PSUM") as pskv, \
                tc.tile_pool(name="pst", bufs=4, space="PSUM") as pst:
            chunks = {}

            def load_chunk(tb):
                t = xA.tile([128, CT, 128], BF16, tag="xch", name="xch")
                nc.sync.dma_start(out=t, in_=xt[tb])
                chunks[tb] = t

            # small per-token tensors at the head of the sync queue so they
            # land before the bulk x/weight traffic
            wgb_sb = wA.tile([128, GC], F32)
            nc.sync.dma_start(out=wgb_sb, in_=_bcast(wgate[:, 0], 0, 128))
            xg_sb = wA.tile([128, TB, GC], BF16)
            nc.sync.dma_start(out=xg_sb, in_=xg[:])
            cos_sb = wA.tile([128, TB, 64], BF16)
            nc.sync.dma_start(out=cos_sb, in_=cosn[:])
            sin_sb = wA.tile([128, TB, 64], BF16)
            nc.sync.dma_start(out=sin_sb, in_=sinn[:])
            ve_sb = wA.tile([128, TB, HD], BF16)
            nc.sync.dma_start(out=ve_sb, in_=ve2[:])
            wq_sb = wA.tile([128, CT, NHL * HD], BF16)
            wkv_sb = wA.tile([128, CT, 2 * HD], BF16)
            nc.gpsimd.dma_start(out=wq_sb, in_=wq[:])
            nc.gpsimd.dma_start(out=wkv_sb, in_=wkv[:])
            load_chunk(0)
            load_chunk(1)
            build_masks()
            load_chunk(2)
            load_chunk(3)

            pend = deque()  # (qhat, khat, t0) awaiting transpose into qT/kT
            hat_pend = deque()  # (qh, kh, rq, qhat, khat) scalar muls deferred

            def emit_hats(qh, kh, rq, qhat, khat):
                for h5 in range(NHL):
                    nc.scalar.mul(
                        qhat[:, h5 * HD:(h5 + 1) * HD],
                        qh[:, h5 * HD:(h5 + 1) * HD], rq[:, h5:h5 + 1])
                nc.scalar.mul(khat, kh, rq[:, NHL:NHL + 1])

            def emit_transposes(pqh, pkh, pt0):
                for hh in range(NHL):
                    tq_ps = pst.tile([128, 128], F32R, tag="tps", name="tps")
                    nc.tensor.transpose(
                        tq_ps, pqh[:, hh * 128:(hh + 1) * 128], identr)
                    nc.scalar.copy(out=qT[:, hh, pt0:pt0 + 128], in_=tq_ps)
                tk_ps = pst.tile([128, 128], F32R, tag="tps", name="tps")
                nc.tensor.transpose(tk_ps, pkh, identr)
                nc.scalar.copy(out=kT[:, pt0:pt0 + 128], in_=tk_ps)

            for tb in range(TB):
                t0 = tb * 128
                if tb + 4 < TB:
                    load_chunk(tb + 4)
                xcht = chunks.pop(tb)
                q_ps = psq.tile([128, NHL * HD], F32, tag="qps")
                kv_ps = pskv.tile([128, 2 * HD], F32, tag="kvps")
                for ct in range(CT):
                    xts = xcht[:, ct, :]
                    nc.tensor.matmul(
                        q_ps, lhsT=xts, rhs=wq_sb[:, ct, :],
                        start=(ct == 0), stop=(ct == CT - 1))
                    nc.tensor.matmul(
                        kv_ps, lhsT=xts, rhs=wkv_sb[:, ct, :],
                        start=(ct == 0), stop=(ct == CT - 1))

                # four-blocks-ago qhat/khat -> qT/kT (PE transposes)
                if len(pend) >= 4:
                    emit_transposes(*pend.popleft())

                # free the PSUM banks right away: q/kv -> SBUF on the
                # scalar engine so the next block's matmuls never wait
                qsb = sbA.tile([128, NHL * HD], F32, tag="qsb")
                nc.scalar.copy(out=qsb, in_=q_ps)
                kvsb = sbA.tile([128, 2 * HD], F32, tag="kvsb")
                nc.scalar.copy(out=kvsb, in_=kv_ps)
                if hat_pend:
                    emit_hats(*hat_pend.popleft())
                # gate = sigmoid(x[:, :32] @ wgate);  v = v_mm + gate * (2*ve)
                zg_sb = sbA.tile([128, 1], F32, tag="zg")
                zscr = sbA.tile([128, GC], F32, tag="zscr")
                nc.vector.scalar_tensor_tensor(
                    out=zscr, in0=xg_sb[:, tb, :], scalar=1.0, in1=wgb_sb,
                    op0=ALU.bypass, op1=ALU.mult, accum_out=zg_sb)
                g_sb = sbA.tile([128, 1], F32, tag="gsb")
                nc.scalar.activation(g_sb, zg_sb, AF.Sigmoid)
                nc.vector.scalar_tensor_tensor(
                    out=vS[:, tb, :], in0=ve_sb[:, tb, :], scalar=g_sb,
                    in1=kvsb[:, HD:2 * HD], op0=ALU.mult, op1=ALU.add)

                # ---- RoPE on q (4 heads batched) and k ----
                cosB = _bcast(cos_sb[:, tb, :], 1, NHL)
                sinB = _bcast(sin_sb[:, tb, :], 1, NHL)
                qv = qsb.rearrange("p (h d) -> p h d", h=NHL)
                qh = sbA.tile([128, NHL * HD], F32, tag="qh", bufs=2)
                qhv = qh.rearrange("p (h d) -> p h d", h=NHL)
                tmp = sbA.tile([128, NHL, 64], F32, tag="tmp")
                nc.vector.tensor_tensor(
                    out=qhv[:, :, 0:64], in0=qv[:, :, 0:64], in1=cosB, op=ALU.mult)
                nc.vector.tensor_tensor(
                    out=tmp, in0=qv[:, :, 64:128], in1=sinB, op=ALU.mult)
                nc.vector.tensor_tensor(
                    out=qhv[:, :, 0:64], in0=qhv[:, :, 0:64], in1=tmp, op=ALU.add)
                nc.vector.tensor_tensor(
                    out=qhv[:, :, 64:128], in0=qv[:, :, 64:128], in1=cosB, op=ALU.mult)
                nc.vector.tensor_tensor(
                    out=tmp, in0=qv[:, :, 0:64], in1=sinB, op=ALU.mult)
                nc.vector.tensor_tensor(
                    out=qhv[:, :, 64:128], in0=qhv[:, :, 64:128], in1=tmp,
                    op=ALU.subtract)
                kv = kvsb[:, 0:HD]
                kh = sbA.tile([128, HD], F32, tag="kh", bufs=2)
                ktmp = sbA.tile([128, 64], F32, tag="ktmp")
                cs1 = cos_sb[:, tb, :]
                sn1 = sin_sb[:, tb, :]
                nc.vector.tensor_tensor(
                    out=kh[:, 0:64], in0=kv[:, 0:64], in1=cs1, op=ALU.mult)
                nc.vector.tensor_tensor(
                    out=ktmp, in0=kv[:, 64:128], in1=sn1, op=ALU.mult)
                nc.vector.tensor_tensor(
                    out=kh[:, 0:64], in0=kh[:, 0:64], in1=ktmp, op=ALU.add)
                nc.vector.tensor_tensor(
                    out=kh[:, 64:128], in0=kv[:, 64:128], in1=cs1, op=ALU.mult)
                nc.vector.tensor_tensor(
                    out=ktmp, in0=kv[:, 0:64], in1=sn1, op=ALU.mult)
                nc.vector.tensor_tensor(
                    out=kh[:, 64:128], in0=kh[:, 64:128], in1=ktmp, op=ALU.subtract)

                # ---- RMSNorm scales for q heads + k in one [128, 5] batch ----
                sq2 = sbA.tile([128, NHL * HD], F32, tag="sq2")
                red = sbA.tile([128, NHL + 1], F32, tag="red")
                for h5 in range(NHL):
                    nc.vector.scalar_tensor_tensor(
                        out=sq2[:, h5 * HD:(h5 + 1) * HD],
                        in0=qh[:, h5 * HD:(h5 + 1) * HD], scalar=1.0,
                        in1=qh[:, h5 * HD:(h5 + 1) * HD], op0=ALU.bypass,
                        op1=ALU.mult, accum_out=red[:, h5:h5 + 1])
                ksq = sbA.tile([128, HD], F32, tag="ksq")
                nc.vector.scalar_tensor_tensor(
                    out=ksq, in0=kh, scalar=1.0, in1=kh, op0=ALU.bypass,
                    op1=ALU.mult, accum_out=red[:, NHL:NHL + 1])
                # m = mean + eps;  rsqrt(m) via bit-trick seed + 2 Newton steps
                # (all on DVE: keeps the scalar engine on a single act table)
                nc.vector.tensor_scalar(
                    out=red, in0=red, scalar1=1.0 / 128.0, scalar2=EPS,
                    op0=ALU.mult, op1=ALU.add)
                rq = sbA.tile([128, NHL + 1], F32, tag="rq", bufs=2)
                rqi = rq.bitcast(I32)
                nc.vector.tensor_scalar(
                    out=rqi, in0=red.bitcast(I32), scalar1=1, scalar2=None,
                    op0=ALU.logical_shift_right)
                nc.vector.tensor_scalar(
                    out=rqi, in0=rqi, scalar1=-1, scalar2=RSQRT_MAGIC,
                    op0=ALU.mult, op1=ALU.add)
                nt = sbA.tile([128, NHL + 1], F32, tag="nt")
                for _ in range(1):
                    nc.vector.tensor_tensor(out=nt, in0=rq, in1=rq, op=ALU.mult)
                    nc.vector.tensor_tensor(out=nt, in0=nt, in1=red, op=ALU.mult)
                    nc.vector.tensor_scalar(
                        out=nt, in0=nt, scalar1=-0.5, scalar2=1.5,
                        op0=ALU.mult, op1=ALU.add)
                    nc.vector.tensor_tensor(out=rq, in0=rq, in1=nt, op=ALU.mult)

                qhat = qkh.tile([128, NHL * HD], F32R, tag="qhat")
                for h5 in range(NHL):
                    nc.scalar.mul(
                        qhat[:, h5 * HD:(h5 + 1) * HD],
                        qh[:, h5 * HD:(h5 + 1) * HD], rq[:, h5:h5 + 1])
                khat = qkh.tile([128, HD], F32R, tag="khat")
                nc.scalar.mul(khat, kh, rq[:, NHL:NHL + 1])
                pend.append((qhat, khat, t0))

            while hat_pend:
                emit_hats(*hat_pend.popleft())
            while pend:
                emit_transposes(*pend.popleft())

        # ---------------- phase B+C: attention + output projection ---------
        wC = stk.enter_context(tc.tile_pool(name="wC", bufs=1))
        wp_sb = wC.tile([128, NHL, OC, 512], BF16)
        nc.sync.dma_start(out=wp_sb, in_=wproj[:])

        with nc.named_scope("phaseBC"), \
                tc.tile_pool(name="ptB", bufs=4) as ptB, \
                tc.tile_pool(name="smB", bufs=2) as smB, \
                tc.tile_pool(name="sbC", bufs=2) as sbC, \
                tc.tile_pool(name="psS", bufs=2, space="PSUM") as psS, \
                tc.tile_pool(name="psy", bufs=2, space="PSUM") as psy, \
                tc.tile_pool(name="psd", bufs=2, space="PSUM") as psd, \
                tc.tile_pool(name="psC", bufs=2, space="PSUM") as psC:

            av_pend = deque()  # closures: AV+den matmuls (and chunk tails)

            def drain(n):
                while len(av_pend) > n:
                    av_pend.popleft()()

            def emit_c_chunk(c):
                drain(0)
                for tb in range(4 * c, 4 * c + 4):
                    t0 = tb * 128
                    o_sb = sbC.tile([128, C], F32, tag="osb", name="osb")
                    for oc in range(OC):
                        o_ps = psC.tile([128, 512], F32, tag="ops", name="ops")
                        for hh in range(NHL):
                            nc.tensor.matmul(
                                o_ps, lhsT=yT[:, hh, t0:t0 + 128],
                                rhs=wp_sb[:, hh, oc, :],
                                start=(hh == 0), stop=(hh == NHL - 1))
                        dst = o_sb[:, oc * 512:(oc + 1) * 512]
                        if oc % 2 == 0:
                            nc.scalar.copy(out=dst, in_=o_ps)
                        else:
                            nc.vector.tensor_copy(out=dst, in_=o_ps)
                        if oc == 1:
                            nc.sync.dma_start(
                                out=out[t0:t0 + 128, 0:1024], in_=o_sb[:, 0:1024])
                    nc.gpsimd.dma_start(
                        out=out[t0:t0 + 128, 1024:2048], in_=o_sb[:, 1024:2048])

            for c in range(NCH):
                i0 = c * 512
                for hh in range(NHL):
                    if hh == 1 and c >= 1:
                        emit_c_chunk(c - 1)
                    yps = psy.tile([128, 512], F32, tag="yps", name="yps")
                    dps = psd.tile([128, 512], F32, tag="dps", name="dps")
                    njb = 4 * c + 4
                    for jb in range(njb):
                        r = jb - 4 * c
                        io2 = 0 if r < 0 else min(128 * r, 256)
                        sps = psS.tile([128, 512], F32, tag="sps", name="sps")
                        nc.tensor.matmul(
                            sps[:, io2:512],
                            lhsT=kT[:, jb * 128:(jb + 1) * 128],
                            rhs=qT[:, hh, i0 + io2:i0 + 512],
                            start=True, stop=True)
                        pt = ptB.tile([128, 512], F32R, tag="pt", name="pt")
                        nc.scalar.activation(
                            pt[:, io2:512], sps[:, io2:512], AF.Exp, scale=ISQ)
                        if r >= 0:
                            nc.vector.tensor_tensor(
                                out=pt[:, io2:512], in0=pt[:, io2:512],
                                in1=masks[:, r, io2:512], op=ALU.mult)

                        def av(jb=jb, pt=pt, io2=io2, yps=yps, dps=dps,
                               first=(jb == 0), last=(jb == njb - 1),
                               hh=hh, i0=i0):
                            nc.tensor.matmul(
                                yps[:, io2:512], lhsT=vS[:, jb, :],
                                rhs=pt[:, io2:512], start=first, stop=last)
                            nc.tensor.matmul(
                                dps[:, io2:512], lhsT=ones128,
                                rhs=pt[:, io2:512], start=first, stop=last)
                            if last:
                                rcp = smB.tile([128, 512], F32, tag="rcp",
                                               name="rcp")
                                nc.vector.reciprocal_approx_fast(
                                    out=rcp, in_=dps)
                                nc.vector.tensor_tensor(
                                    out=yT[:, hh, i0:i0 + 512], in0=yps,
                                    in1=rcp, op=ALU.mult)

                        av_pend.append(av)
                        drain(2)
            drain(0)
            emit_c_chunk(NCH - 1)

    nc.compile()
    return nc


_NC_CACHE = {}


def get_nc(T=2048):
    if T not in _NC_CACHE:
        _NC_CACHE[T] = build(T)
    return _NC_CACHE[T]


def make_in_maps(x, ve, cos, sin, Wq, Wk, Wv, Wproj, Wgate):
    """Shard full inputs into 8 per-core input maps (2 batch x 4 head groups)."""
    import ml_dtypes
    bf16 = ml_dtypes.bfloat16
    x = np.asarray(x, np.float32)
    ve = np.asarray(ve, np.float32)
    cosn = np.asarray(cos, np.float32)[0, :, 0, :]
    sinn = np.asarray(sin, np.float32)[0, :, 0, :]
    TT = cosn.shape[0]
    cosn = np.ascontiguousarray(
        cosn.astype(bf16).reshape(TT // 128, 128, 64).transpose(1, 0, 2))
    sinn = np.ascontiguousarray(
        sinn.astype(bf16).reshape(TT // 128, 128, 64).transpose(1, 0, 2))
    Wq = np.asarray(Wq, np.float32)
    Wk = np.asarray(Wk, np.float32)
    Wv = np.asarray(Wv, np.float32)
    Wproj = np.asarray(Wproj, np.float32)
    Wgate = np.asarray(Wgate, np.float32)
    T = x.shape[1]
    TB, CT = T // 128, C // 128
    # [TB, p, ct, t] tiling of x (partition p = channel within ct-block)
    xts = [np.ascontiguousarray(
        x[b].astype(bf16).reshape(TB, 128, CT, 128).transpose(0, 3, 2, 1))
        for b in range(B)]
    xgs = [np.ascontiguousarray(
        x[b][:, :GC].astype(bf16).reshape(TB, 128, GC).transpose(1, 0, 2))
        for b in range(B)]
    in_maps = []
    for core in range(8):
        b, g = divmod(core, 4)
        in_maps.append({
            "xt": xts[b],
            "xg": xgs[b],
            "wq": np.ascontiguousarray(
                Wq[:, g * 512:(g + 1) * 512].astype(bf16)
                .reshape(CT, 128, 512).transpose(1, 0, 2)),
            "wkv": np.ascontiguousarray(np.concatenate(
                [Wk[:, g * 128:(g + 1) * 128].astype(bf16)
                 .reshape(CT, 128, 128),
                 Wv[:, g * 128:(g + 1) * 128].astype(bf16)
                 .reshape(CT, 128, 128)], axis=2).transpose(1, 0, 2)),
            "wproj": np.ascontiguousarray(
                Wproj[g * 512:(g + 1) * 512, :].astype(bf16)
                .reshape(4, 128, 4, 512).transpose(1, 0, 2, 3)),
            "wgate": np.ascontiguousarray(Wgate[:, g:g + 1]),
            "ve2": np.ascontiguousarray(
                (2.0 * ve[b][:, g * 128:(g + 1) * 128]).astype(bf16)
                .reshape(TB, 128, 128).transpose(1, 0, 2)),
            "cosn": cosn,
            "sinn": sinn,
        })
    return in_maps


def run_cores(in_maps, trace=False, **kw):
    nc = get_nc(in_maps[0]["xt"].shape[0] * 128)
    return run_bass_kernel_spmd(nc, in_maps, core_ids=list(range(8)), trace=trace, **kw)


def kernel(**inputs):
    in_maps = make_in_maps(
        inputs["x"], inputs["ve"], inputs["cos"], inputs["sin"],
        inputs["Wq"], inputs["Wk"], inputs["Wv"], inputs["Wproj"], inputs["Wgate"])
    res = run_cores(in_maps)
    parts = [res.results[i]["out"] for i in range(8)]
    out = np.stack([
        parts[0] + parts[1] + parts[2] + parts[3],
        parts[4] + parts[5] + parts[6] + parts[7],
    ]).astype(np.float32)
    return out


# revision 36
# speedup vs baseline: 1.4591x; 1.0198x over previous
"""Trainium2 Bass kernel for a causal self-attention block (GQA + gated value
embedding + RoPE + QK-RMSNorm), sharded over 8 NeuronCores.

Sharding: 8 cores = 2 (batch) x 4 (kv-head groups).  Each core computes, for
its batch b and head-group g (4 q-heads + 1 kv-head):
    q/k/v projections, gated ve addition, RoPE, RMSNorm, causal attention,
    and the partial output projection  y_g @ Wproj[g*512:(g+1)*512, :].
The host sums the 4 per-group partials for each batch (the Wproj
contraction distributes over head groups).

x is transposed on the host (xT), so the qkv matmuls consume it directly as
the stationary operand with no on-chip transposes.  The softmax denominator
is accumulated through an all-ones [128,128] stationary matmul, which makes
the PSUM result partition-replicated so the normalization needs no
cross-partition broadcast.  The output projection is interleaved into the
attention loop chunk-by-chunk to keep the PE warm.

Self-contained: hardcodes shapes; accepts FULL inputs, returns FULL output.
"""

from collections import deque
from contextlib import ExitStack

import numpy as np

import concourse.bacc as bacc
import concourse.bass as bass
import concourse.mybir as mybir
import concourse.tile as tile
from concourse.bass_utils import run_bass_kernel_spmd
from concourse.masks import make_identity

F32 = mybir.dt.float32
F32R = mybir.dt.float32r
BF16 = mybir.dt.bfloat16
I32 = mybir.dt.int32
AF = mybir.ActivationFunctionType
ALU = mybir.AluOpType
AX = mybir.AxisListType

B, C, HD, NHL, GC = 2, 2048, 128, 4, 32  # NHL = local q heads per core
EPS = float(np.finfo(np.float32).eps)
ISQ = 1.0 / float(np.sqrt(128.0))
RSQRT_MAGIC = 0x5F3759DF


def _bcast(ap_, idx, count):
    """Insert a step-0 (broadcast) dim at position idx of the AP dims."""
    lst = [list(p) for p in ap_.ap]
    lst.insert(idx, [0, count])
    return bass.AP(ap_.tensor, ap_.offset, lst)


def build(T=2048):
    TB = T // 128   # token blocks
    CT = C // 128   # contraction tiles for qkv
    NCH = T // 512  # i-chunks for attention
    OC = C // 512   # output chunks for proj

    nc = bacc.Bacc("TRN2", target_bir_lowering=False, debug=False)
    # all inputs host-tiled to [128 partitions, ...contiguous] so every DMA
    # needs only 128 descriptors (one 4-16KB segment per partition)
    xt = nc.dram_tensor("xt", [TB, 128, CT, 128], BF16, kind="ExternalInput")
    xg = nc.dram_tensor("xg", [128, TB, GC], BF16, kind="ExternalInput")
    wq = nc.dram_tensor("wq", [128, CT, NHL * HD], BF16, kind="ExternalInput")
    wkv = nc.dram_tensor("wkv", [128, CT, 2 * HD], BF16, kind="ExternalInput")
    wproj = nc.dram_tensor("wproj", [128, NHL, OC, 512], BF16,
                           kind="ExternalInput")
    wgate = nc.dram_tensor("wgate", [GC, 1], F32, kind="ExternalInput")
    ve2 = nc.dram_tensor("ve2", [128, TB, HD], BF16, kind="ExternalInput")
    cosn = nc.dram_tensor("cosn", [128, TB, 64], BF16, kind="ExternalInput")
    sinn = nc.dram_tensor("sinn", [128, TB, 64], BF16, kind="ExternalInput")
    out = nc.dram_tensor("out", [T, C], F32, kind="ExternalOutput")

    with ExitStack() as stk:
        tc = stk.enter_context(tile.TileContext(nc))
        gpool = stk.enter_context(tc.tile_pool(name="gconst", bufs=1))
        identw = gpool.tile([128, 128], F32)
        nc.gpsimd.memset(identw, 0.0)
        ones_g = gpool.tile([128, 1], F32)
        nc.gpsimd.memset(ones_g, 1.0)
        nc.gpsimd.affine_select(
            out=identw, in_=_bcast(ones_g[:, 0], 1, 128), pattern=[[1, 128]],
            compare_op=ALU.is_equal, fill=0.0, base=0, channel_multiplier=-1)
        ident = gpool.tile([128, 128], F32)
        make_identity(nc, ident)
        identr = gpool.tile([128, 128], F32R)
        nc.vector.tensor_copy(out=identr, in_=ident)
        ones_f = gpool.tile([128, 512], F32)
        ones128 = gpool.tile([128, 128], F32R)
        mask_f = gpool.tile([128, 4, 512], F32)
        masks = gpool.tile([128, 4, 512], F32R)

        def build_masks():
            # static causal masks for the 4 diagonal positions: mask[r][p,col]
            # = 1 iff col >= 128*r + p (col = q index within its 512-chunk,
            # p = k%128).  Emitted after phase A's DMA issues so the gpsimd
            # queue serves the weight DMAs first.
            nc.vector.memset(ones_f, 1.0)
            nc.vector.tensor_copy(out=ones128, in_=ones_f[:, 0:128])
            nc.vector.memset(mask_f, 0.0)
            for r in range(4):
                nc.gpsimd.affine_select(
                    out=mask_f[:, r, 128 * r:512],
                    in_=ones_f[:, 0:512 - 128 * r],
                    pattern=[[1, 512 - 128 * r]], compare_op=ALU.is_ge,
                    fill=0.0, base=0, channel_multiplier=-1)
            nc.vector.tensor_copy(out=masks, in_=mask_f)

        # PE warmup: dummy transposes so HAM reaches full clock while the
        # first DMAs land.
        with tc.tile_pool(name="warm", bufs=2, space="PSUM") as warm:
            for _ in range(12):
                w_ps = warm.tile([128, 128], F32R, tag="wps", name="wps")
                nc.tensor.transpose(w_ps, identr, identr)

        persist = stk.enter_context(tc.tile_pool(name="persist", bufs=1))
        qT = persist.tile([128, NHL, T], BF16)   # [d, h, t]
        kT = persist.tile([128, T], BF16)        # [d, t]
        vS = persist.tile([128, TB, HD], F32R)   # [t%128, t//128, d]
        yT = persist.tile([128, NHL, T], BF16)   # [d, h, t]

        # ---------------- phase A: qkv + rope + rmsnorm --------------------
        with nc.named_scope("phaseA"), \
                tc.tile_pool(name="wA", bufs=1) as wA, \
                tc.tile_pool(name="xA", bufs=4) as xA, \
                tc.tile_pool(name="sbA", bufs=1) as sbA, \
                tc.tile_pool(name="qkh", bufs=5) as qkh, \
                tc.tile_pool(name="psq", bufs=2, space="PSUM") as psq, \
                tc.tile_pool(name="pskv", bufs=2, space="PSUM") as pskv, \
                tc.tile_pool(name="pst", bufs=4, space="PSUM") as pst:
            chunks = {}

            def load_chunk(tb):
                t = xA.tile([128, CT, 128], BF16, tag="xch", name="xch")
                nc.sync.dma_start(out=t, in_=xt[tb])
                chunks[tb] = t

            # first x chunk, then the small per-token tensors, on the sync
            # queue ahead of the bulk x/weight traffic
            load_chunk(0)
            wgb_sb = wA.tile([128, GC], F32)
            nc.sync.dma_start(out=wgb_sb, in_=_bcast(wgate[:, 0], 0, 128))
            xg_sb = wA.tile([128, TB, GC], BF16)
            nc.sync.dma_start(out=xg_sb, in_=xg[:])
            cos_sb = wA.tile([128, TB, 64], BF16)
            nc.sync.dma_start(out=cos_sb, in_=cosn[:])
            sin_sb = wA.tile([128, TB, 64], BF16)
            nc.sync.dma_start(out=sin_sb, in_=sinn[:])
            ve_sb = wA.tile([128, TB, HD], BF16)
            nc.sync.dma_start(out=ve_sb, in_=ve2[:])
            wq_sb = wA.tile([128, CT, NHL * HD], BF16)
            wkv_sb = wA.tile([128, CT, 2 * HD], BF16)
            nc.gpsimd.dma_start(out=wq_sb, in_=wq[:])
            nc.gpsimd.dma_start(out=wkv_sb, in_=wkv[:])
            load_chunk(1)
            build_masks()
            load_chunk(2)
            load_chunk(3)

            pend = deque()  # (qhat, khat, t0) awaiting transpose into qT/kT
            hat_pend = deque()  # (qh, kh, rq, qhat, khat) scalar muls deferred

            def emit_hats(qh, kh, rq, qhat, khat):
                for h5 in range(NHL):
                    nc.scalar.mul(
                        qhat[:, h5 * HD:(h5 + 1) * HD],
                        qh[:, h5 * HD:(h5 + 1) * HD], rq[:, h5:h5 + 1])
                nc.scalar.mul(khat, kh, rq[:, NHL:NHL + 1])

            def emit_transposes(pqh, pkh, pt0):
                for hh in range(NHL):
                    tq_ps = pst.tile([128, 128], F32R, tag="tps", name="tps")
                    nc.tensor.transpose(
                        tq_ps, pqh[:, hh * 128:(hh + 1) * 128], identr)
                    nc.scalar.copy(out=qT[:, hh, pt0:pt0 + 128], in_=tq_ps)
                tk_ps = pst.tile([128, 128], F32R, tag="tps", name="tps")
                nc.tensor.transpose(tk_ps, pkh, identr)
                nc.scalar.copy(out=kT[:, pt0:pt0 + 128], in_=tk_ps)

            for tb in range(TB):
                t0 = tb * 128
                if tb + 4 < TB:
                    load_chunk(tb + 4)
                xcht = chunks.pop(tb)
                q_ps = psq.tile([128, NHL * HD], F32, tag="qps")
                kv_ps = pskv.tile([128, 2 * HD], F32, tag="kvps")
                for ct in range(CT):
                    xts = xcht[:, ct, :]
                    nc.tensor.matmul(
                        q_ps, lhsT=xts, rhs=wq_sb[:, ct, :],
                        start=(ct == 0), stop=(ct == CT - 1))
                    nc.tensor.matmul(
                        kv_ps, lhsT=xts, rhs=wkv_sb[:, ct, :],
                        start=(ct == 0), stop=(ct == CT - 1))

                # four-blocks-ago qhat/khat -> qT/kT (PE transposes)
                if len(pend) >= 2:
                    emit_transposes(*pend.popleft())

                # free the PSUM banks right away: q/kv -> SBUF on the
                # scalar engine so the next block's matmuls never wait
                qsb = sbA.tile([128, NHL * HD], F32, tag="qsb")
                nc.scalar.copy(out=qsb, in_=q_ps)
                kvsb = sbA.tile([128, 2 * HD], F32, tag="kvsb")
                nc.scalar.copy(out=kvsb, in_=kv_ps)
                if hat_pend:
                    emit_hats(*hat_pend.popleft())
                # gate = sigmoid(x[:, :32] @ wgate);  v = v_mm + gate * (2*ve)
                zg_sb = sbA.tile([128, 1], F32, tag="zg")
                zscr = sbA.tile([128, GC], F32, tag="zscr")
                nc.vector.scalar_tensor_tensor(
                    out=zscr, in0=xg_sb[:, tb, :], scalar=1.0, in1=wgb_sb,
                    op0=ALU.bypass, op1=ALU.mult, accum_out=zg_sb)
                g_sb = sbA.tile([128, 1], F32, tag="gsb")
                nc.scalar.activation(g_sb, zg_sb, AF.Sigmoid)
                nc.vector.scalar_tensor_tensor(
                    out=vS[:, tb, :], in0=ve_sb[:, tb, :], scalar=g_sb,
                    in1=kvsb[:, HD:2 * HD], op0=ALU.mult, op1=ALU.add)

                # ---- RoPE on q (4 heads batched) and k ----
                cosB = _bcast(cos_sb[:, tb, :], 1, NHL)
                sinB = _bcast(sin_sb[:, tb, :], 1, NHL)
                qv = qsb.rearrange("p (h d) -> p h d", h=NHL)
                qh = sbA.tile([128, NHL * HD], F32, tag="qh", bufs=2)
                qhv = qh.rearrange("p (h d) -> p h d", h=NHL)
                tmp = sbA.tile([128, NHL, 64], F32, tag="tmp")
                nc.vector.tensor_tensor(
                    out=qhv[:, :, 0:64], in0=qv[:, :, 0:64], in1=cosB, op=ALU.mult)
                nc.vector.tensor_tensor(
                    out=tmp, in0=qv[:, :, 64:128], in1=sinB, op=ALU.mult)
                nc.vector.tensor_tensor(
                    out=qhv[:, :, 0:64], in0=qhv[:, :, 0:64], in1=tmp, op=ALU.add)
                nc.vector.tensor_tensor(
                    out=qhv[:, :, 64:128], in0=qv[:, :, 64:128], in1=cosB, op=ALU.mult)
                nc.vector.tensor_tensor(
                    out=tmp, in0=qv[:, :, 0:64], in1=sinB, op=ALU.mult)
                nc.vector.tensor_tensor(
                    out=qhv[:, :, 64:128], in0=qhv[:, :, 64:128], in1=tmp,
                    op=ALU.subtract)
                kv = kvsb[:, 0:HD]
                kh = sbA.tile([128, HD], F32, tag="kh", bufs=2)
                ktmp = sbA.tile([128, 64], F32, tag="ktmp")
                cs1 = cos_sb[:, tb, :]
                sn1 = sin_sb[:, tb, :]
                nc.vector.tensor_tensor(
                    out=kh[:, 0:64], in0=kv[:, 0:64], in1=cs1, op=ALU.mult)
                nc.vector.tensor_tensor(
                    out=ktmp, in0=kv[:, 64:128], in1=sn1, op=ALU.mult)
                nc.vector.tensor_tensor(
                    out=kh[:, 0:64], in0=kh[:, 0:64], in1=ktmp, op=ALU.add)
                nc.vector.tensor_tensor(
                    out=kh[:, 64:128], in0=kv[:, 64:128], in1=cs1, op=ALU.mult)
                nc.vector.tensor_tensor(
                    out=ktmp, in0=kv[:, 0:64], in1=sn1, op=ALU.mult)
                nc.vector.tensor_tensor(
                    out=kh[:, 64:128], in0=kh[:, 64:128], in1=ktmp, op=ALU.subtract)

                # ---- RMSNorm scales for q heads + k in one [128, 5] batch ----
                sq2 = sbA.tile([128, NHL * HD], F32, tag="sq2")
                red = sbA.tile([128, NHL + 1], F32, tag="red")
                for h5 in range(NHL):
                    nc.vector.scalar_tensor_tensor(
                        out=sq2[:, h5 * HD:(h5 + 1) * HD],
                        in0=qh[:, h5 * HD:(h5 + 1) * HD], scalar=1.0,
                        in1=qh[:, h5 * HD:(h5 + 1) * HD], op0=ALU.bypass,
                        op1=ALU.mult, accum_out=red[:, h5:h5 + 1])
                ksq = sbA.tile([128, HD], F32, tag="ksq")
                nc.vector.scalar_tensor_tensor(
                    out=ksq, in0=kh, scalar=1.0, in1=kh, op0=ALU.bypass,
                    op1=ALU.mult, accum_out=red[:, NHL:NHL + 1])
                # m = mean + eps;  rsqrt(m) via bit-trick seed + 2 Newton steps
                # (all on DVE: keeps the scalar engine on a single act table)
                nc.vector.tensor_scalar(
                    out=red, in0=red, scalar1=1.0 / 128.0, scalar2=EPS,
                    op0=ALU.mult, op1=ALU.add)
                rq = sbA.tile([128, NHL + 1], F32, tag="rq", bufs=2)
                rqi = rq.bitcast(I32)
                nc.vector.tensor_scalar(
                    out=rqi, in0=red.bitcast(I32), scalar1=1, scalar2=None,
                    op0=ALU.logical_shift_right)
                nc.vector.tensor_scalar(
                    out=rqi, in0=rqi, scalar1=-1, scalar2=RSQRT_MAGIC,
                    op0=ALU.mult, op1=ALU.add)
                nt = sbA.tile([128, NHL + 1], F32, tag="nt")
                for _ in range(1):
                    nc.vector.tensor_tensor(out=nt, in0=rq, in1=rq, op=ALU.mult)
                    nc.vector.tensor_tensor(out=nt, in0=nt, in1=red, op=ALU.mult)
                    nc.vector.tensor_scalar(
                        out=nt, in0=nt, scalar1=-0.5, scalar2=1.5,
                        op0=ALU.mult, op1=ALU.add)
                    nc.vector.tensor_tensor(out=rq, in0=rq, in1=nt, op=ALU.mult)

                qhat = qkh.tile([128, NHL * HD], F32R, tag="qhat")
                for h5 in range(NHL):
                    nc.scalar.mul(
                        qhat[:, h5 * HD:(h5 + 1) * HD],
                        qh[:, h5 * HD:(h5 + 1) * HD], rq[:, h5:h5 + 1])
                khat = qkh.tile([128, HD], F32R, tag="khat")
                nc.scalar.mul(khat, kh, rq[:, NHL:NHL + 1])
                pend.append((qhat, khat, t0))

            while hat_pend:
                emit_hats(*hat_pend.popleft())
            while pend:
                emit_transposes(*pend.popleft())

        # ---------------- phase B+C: attention + output projection ---------
        wC = stk.enter_context(tc.tile_pool(name="wC", bufs=1))
        wp_sb = wC.tile([128, NHL, OC, 512], BF16)
        nc.sync.dma_start(out=wp_sb, in_=wproj[:])

        with nc.named_scope("phaseBC"), \
                tc.tile_pool(name="ptB", bufs=4) as ptB, \
                tc.tile_pool(name="smB", bufs=2) as smB, \
                tc.tile_pool(name="sbC", bufs=2) as sbC, \
                tc.tile_pool(name="psS", bufs=2, space="PSUM") as psS, \
                tc.tile_pool(name="psy", bufs=2, space="PSUM") as psy, \
                tc.tile_pool(name="psd", bufs=2, space="PSUM") as psd, \
                tc.tile_pool(name="psC", bufs=2, space="PSUM") as psC:

            av_pend = deque()  # closures: AV+den matmuls (and chunk tails)

            def drain(n):
                while len(av_pend) > n:
                    av_pend.popleft()()

            def emit_c_chunk(c):
                drain(0)
                for tb in range(4 * c, 4 * c + 4):
                    t0 = tb * 128
                    o_sb = sbC.tile([128, C], F32, tag="osb", name="osb")
                    for oc in range(OC):
                        o_ps = psC.tile([128, 512], F32, tag="ops", name="ops")
                        for hh in range(NHL):
                            nc.tensor.matmul(
                                o_ps, lhsT=yT[:, hh, t0:t0 + 128],
                                rhs=wp_sb[:, hh, oc, :],
                                start=(hh == 0), stop=(hh == NHL - 1))
                        dst = o_sb[:, oc * 512:(oc + 1) * 512]
                        if oc % 2 == 0:
                            nc.scalar.copy(out=dst, in_=o_ps)
                        else:
                            nc.vector.tensor_copy(out=dst, in_=o_ps)
                        if oc == 1:
                            nc.sync.dma_start(
                                out=out[t0:t0 + 128, 0:1024], in_=o_sb[:, 0:1024])
                    nc.gpsimd.dma_start(
                        out=out[t0:t0 + 128, 1024:2048], in_=o_sb[:, 1024:2048])

            for c in range(NCH):
                i0 = c * 512
                for hh in range(NHL):
                    if hh == 1 and c >= 1:
                        emit_c_chunk(c - 1)
                    yps = psy.tile([128, 512], F32, tag="yps", name="yps")
                    dps = psd.tile([128, 512], F32, tag="dps", name="dps")
                    njb = 4 * c + 4
                    for jb in range(njb):
                        r = jb - 4 * c
                        io2 = 0 if r < 0 else min(128 * r, 256)
                        sps = psS.tile([128, 512], F32, tag="sps", name="sps")
                        nc.tensor.matmul(
                            sps[:, io2:512],
                            lhsT=kT[:, jb * 128:(jb + 1) * 128],
                            rhs=qT[:, hh, i0 + io2:i0 + 512],
                            start=True, stop=True)
                        pt = ptB.tile([128, 512], F32R, tag="pt", name="pt")
                        nc.scalar.activation(
                            pt[:, io2:512], sps[:, io2:512], AF.Exp, scale=ISQ)
                        if r >= 0:
                            nc.vector.tensor_tensor(
                                out=pt[:, io2:512], in0=pt[:, io2:512],
                                in1=masks[:, r, io2:512], op=ALU.mult)

                        def av(jb=jb, pt=pt, io2=io2, yps=yps, dps=dps,
                               first=(jb == 0), last=(jb == njb - 1),
                               hh=hh, i0=i0):
                            nc.tensor.matmul(
                                yps[:, io2:512], lhsT=vS[:, jb, :],
                                rhs=pt[:, io2:512], start=first, stop=last)
                            nc.tensor.matmul(
                                dps[:, io2:512], lhsT=ones128,
                                rhs=pt[:, io2:512], start=first, stop=last)
                            if last:
                                rcp = smB.tile([128, 512], F32, tag="rcp",
                                               name="rcp")
                                nc.vector.reciprocal_approx_fast(
                                    out=rcp, in_=dps)
                                nc.vector.tensor_tensor(
                                    out=yT[:, hh, i0:i0 + 512], in0=yps,
                                    in1=rcp, op=ALU.mult)

                        av_pend.append(av)
                        drain(2)
            drain(0)
            emit_c_chunk(NCH - 1)

    nc.compile()
    return nc


_NC_CACHE = {}


def get_nc(T=2048):
    if T not in _NC_CACHE:
        _NC_CACHE[T] = build(T)
    return _NC_CACHE[T]


def make_in_maps(x, ve, cos, sin, Wq, Wk, Wv, Wproj, Wgate):
    """Shard full inputs into 8 per-core input maps (2 batch x 4 head groups)."""
    import ml_dtypes
    bf16 = ml_dtypes.bfloat16
    x = np.asarray(x, np.float32)
    ve = np.asarray(ve, np.float32)
    cosn = np.asarray(cos, np.float32)[0, :, 0, :]
    sinn = np.asarray(sin, np.float32)[0, :, 0, :]
    TT = cosn.shape[0]
    cosn = np.ascontiguousarray(
        cosn.astype(bf16).reshape(TT // 128, 128, 64).transpose(1, 0, 2))
    sinn = np.ascontiguousarray(
        sinn.astype(bf16).reshape(TT // 128, 128, 64).transpose(1, 0, 2))
    Wq = np.asarray(Wq, np.float32)
    Wk = np.asarray(Wk, np.float32)
    Wv = np.asarray(Wv, np.float32)
    Wproj = np.asarray(Wproj, np.float32)
    Wgate = np.asarray(Wgate, np.float32)
    T = x.shape[1]
    TB, CT = T // 128, C // 128
    # [TB, p, ct, t] tiling of x (partition p = channel within ct-block)
    xts = [np.ascontiguousarray(
        x[b].astype(bf16).reshape(TB, 128, CT, 128).transpose(0, 3, 2, 1))
        for b in range(B)]
    xgs = [np.ascontiguousarray(
        x[b][:, :GC].astype(bf16).reshape(TB, 128, GC).transpose(1, 0, 2))
        for b in range(B)]
    in_maps = []
    for core in range(8):
        b, g = divmod(core, 4)
        in_maps.append({
            "xt": xts[b],
            "xg": xgs[b],
            "wq": np.ascontiguousarray(
                Wq[:, g * 512:(g + 1) * 512].astype(bf16)
                .reshape(CT, 128, 512).transpose(1, 0, 2)),
            "wkv": np.ascontiguousarray(np.concatenate(
                [Wk[:, g * 128:(g + 1) * 128].astype(bf16)
                 .reshape(CT, 128, 128),
                 Wv[:, g * 128:(g + 1) * 128].astype(bf16)
                 .reshape(CT, 128, 128)], axis=2).transpose(1, 0, 2)),
            "wproj": np.ascontiguousarray(
                Wproj[g * 512:(g + 1) * 512, :].astype(bf16)
                .reshape(4, 128, 4, 512).transpose(1, 0, 2, 3)),
            "wgate": np.ascontiguousarray(Wgate[:, g:g + 1]),
            "ve2": np.ascontiguousarray(
                (2.0 * ve[b][:, g * 128:(g + 1) * 128]).astype(bf16)
                .reshape(TB, 128, 128).transpose(1, 0, 2)),
            "cosn": cosn,
            "sinn": sinn,
        })
    return in_maps


def run_cores(in_maps, trace=False, **kw):
    nc = get_nc(in_maps[0]["xt"].shape[0] * 128)
    return run_bass_kernel_spmd(nc, in_maps, core_ids=list(range(8)), trace=trace, **kw)


def kernel(**inputs):
    in_maps = make_in_maps(
        inputs["x"], inputs["ve"], inputs["cos"], inputs["sin"],
        inputs["Wq"], inputs["Wk"], inputs["Wv"], inputs["Wproj"], inputs["Wgate"])
    res = run_cores(in_maps)
    parts = [res.results[i]["out"] for i in range(8)]
    out = np.stack([
        parts[0] + parts[1] + parts[2] + parts[3],
        parts[4] + parts[5] + parts[6] + parts[7],
    ]).astype(np.float32)
    return out
